# revision 2
# baseline (speedup 1.0000x reference)
"""Single-head causal attention (B=8, T=2048, D=1024, H=128) on 8 TRN2 NeuronCores.

Sharding: one batch element per core (data-parallel over B).

Per-core algorithm (all big matmuls in float32r: full PE speed, ~1.5e-4 rel err):
  - host supplies x^T [D, T] so the d-contraction has d on partitions
  - Q^T, K^T = W^T @ x^T   [H=128, T] via PE, d-tile-outer to overlap with DMA
  - V^T likewise, then PE-transposed to V [T, H] tiles (PV needs k on partitions)
  - per 512-wide q-chunk: S^T[k, q] = K^T_tile.T @ Q^T_chunk, exp via ACT
    (no max-subtraction: scores are O(20) for this distribution, exp is safe in fp32),
    causal mask on diagonal tiles via shifted upper-tri mask multiply,
    O^T[h, q] += V_tile.T @ P^T accumulated over k-tiles in PSUM,
    row-sums via DVE adds of P^T tiles + ones-matmul, 1/sums broadcast via
    rank-1 matmul, final O^T * (1/sums) on DVE, DMA out.
  - host transposes O^T -> [T, H] per batch.
"""
import numpy as np

B, T, D, H = 8, 2048, 1024, 128
ND = D // 128      # 8 d-tiles
NTK = T // 128     # 16 k-tiles
NCH = T // 512     # 4 q-chunks
SCALE = float(H) ** -0.5

_CACHE = {}


def _build():
    import concourse.bass as bass  # noqa: F401
    from concourse import bacc
    import concourse.mybir as mybir
    import concourse.tile as tile
    from concourse.masks import make_identity

    f32 = mybir.dt.float32
    f32r = mybir.dt.float32r

    nc = bacc.Bacc("TRN2", target_bir_lowering=False)
    xt_d = nc.dram_tensor("xt", (D, T), f32r, kind="ExternalInput")
    wq_d = nc.dram_tensor("wq", (128, ND, H), f32r, kind="ExternalInput")
    wk_d = nc.dram_tensor("wk", (128, ND, H), f32r, kind="ExternalInput")
    wv_d = nc.dram_tensor("wv", (128, ND, H), f32r, kind="ExternalInput")
    ot_d = nc.dram_tensor("ot", (H, T), f32, kind="ExternalOutput")

    with tile.TileContext(nc) as tc:
        with (
            tc.tile_pool(name="sb", bufs=1) as sb,
            tc.tile_pool(name="ps", bufs=1, space="PSUM") as ps,
        ):
            # ---- loads ----
            xt = sb.tile([128, ND, T], f32r, tag="xt")
            # chunk-major subtile DMAs so chunk ch's projections start after ~2MB
            for ch in range(NCH):
                for d in range(ND):
                    nc.sync.dma_start(xt[:, d, ch * 512:(ch + 1) * 512],
                                      xt_d[d * 128:(d + 1) * 128, ch * 512:(ch + 1) * 512])
            wq = sb.tile([128, ND, H], f32r, tag="wq")
            wk = sb.tile([128, ND, H], f32r, tag="wk")
            wv = sb.tile([128, ND, H], f32r, tag="wv")
            nc.sync.dma_start(wq[:], wq_d[:])
            nc.sync.dma_start(wk[:], wk_d[:])
            nc.sync.dma_start(wv[:], wv_d[:])

            # ---- constants ----
            ident = sb.tile([128, 128], f32, tag="ident")
            make_identity(nc, ident[:])
            # mask M[k, col] = 1 iff col - k >= 384; U_m = M[:, (3-m)*128 : +512]
            m32 = sb.tile([128, 896], f32, tag="m32")
            nc.gpsimd.memset(m32[:], 1.0)
            nc.gpsimd.affine_select(
                out=m32[:], in_=m32[:],
                compare_op=mybir.AluOpType.is_ge, fill=0.0,
                base=-384, pattern=[[1, 896]], channel_multiplier=-1,
            )
            maskM = sb.tile([128, 896], f32r, tag="maskM")
            nc.vector.tensor_copy(maskM[:], m32[:])
            ones_c32 = sb.tile([128, 1], f32, tag="ones_c32")
            nc.gpsimd.memset(ones_c32[:], 1.0)
            ones_col = sb.tile([128, 1], f32r, tag="ones_col")
            nc.vector.tensor_copy(ones_col[:], ones_c32[:])
            ones_r32 = sb.tile([1, 128], f32, tag="ones_r32")
            nc.gpsimd.memset(ones_r32[:], 1.0)
            ones_row = sb.tile([1, 128], f32r, tag="ones_row")
            nc.vector.tensor_copy(ones_row[:], ones_r32[:])

            # ---- projections (d-tile outer so PE consumes DMA'd tiles as they land) ----
            qt = sb.tile([128, T], f32r, tag="qt")   # Q^T [h, t]
            kt = sb.tile([128, T], f32r, tag="kt")   # K^T [h, t]
            vt = sb.tile([128, T], f32, tag="vt")    # V^T staging
            v = sb.tile([128, NTK, H], f32r, tag="v")  # V [k, h] tiles

            def attention_chunk(c):
                otp = ps.tile([128, 512], f32, tag="otacc", bufs=2)
                pacc = sb.tile([128, 512], f32r, tag="pacc", bufs=2)
                nk = 4 * c + 4
                for j in range(nk):
                    diag = j >= 4 * c
                    m = j - 4 * c if diag else 0
                    lo = 128 * m  # valid q_local range [lo, 512)
                    stp = ps.tile([128, 512], f32, tag="big", bufs=4)
                    nc.tensor.matmul(
                        stp[:, lo:512],
                        kt[:, j * 128:(j + 1) * 128],
                        qt[:, c * 512 + lo:(c + 1) * 512],
                        start=True, stop=True,
                    )
                    pt = sb.tile([128, 512], f32r, tag="pt", bufs=6)
                    if diag:
                        praw = sb.tile([128, 512], f32r, tag="praw", bufs=3)
                        nc.scalar.activation(
                            praw[:, lo:512], stp[:, lo:512],
                            mybir.ActivationFunctionType.Exp, scale=SCALE)
                        nc.vector.tensor_mul(
                            pt[:, lo:512], praw[:, lo:512], maskM[:, 384:896 - lo])
                    else:
                        nc.scalar.activation(
                            pt[:], stp[:],
                            mybir.ActivationFunctionType.Exp, scale=SCALE)
                    nc.tensor.matmul(
                        otp[:, lo:512], v[:, j, :], pt[:, lo:512],
                        start=(j == 0), stop=(j == nk - 1),
                    )
                    with nc.allow_low_precision(reason="f32r softmax denominator"):
                        if j == 0:
                            nc.vector.tensor_copy(pacc[:], pt[:])
                        else:
                            nc.vector.tensor_add(pacc[:, lo:512], pacc[:, lo:512],
                                                 pt[:, lo:512])
                sums = ps.tile([1, 512], f32, tag="sums")
                nc.tensor.matmul(sums[:], ones_col[:], pacc[:], start=True, stop=True)
                recip = sb.tile([1, 512], f32r, tag="recip", bufs=2)
                with nc.allow_low_precision(reason="f32r softmax denominator"):
                    nc.vector.reciprocal(recip[:], sums[:])
                bc = ps.tile([128, 512], f32, tag="bcast")
                nc.tensor.matmul(bc[:], ones_row[:], recip[:], start=True, stop=True)
                bc_sb = sb.tile([128, 512], f32, tag="bcsb", bufs=2)
                nc.vector.tensor_copy(bc_sb[:], bc[:])
                ot_sb = sb.tile([128, 512], f32, tag="otsb", bufs=2)
                nc.vector.tensor_mul(ot_sb[:], otp[:], bc_sb[:])
                nc.sync.dma_start(ot_d[:, c * 512:(c + 1) * 512], ot_sb[:])


            # ---- fused chunk-major pipeline: proj(ch) -> transposes -> attention(ch)
            for ch in range(NCH):
                for w_sb, dst, dst_r in ((wv, vt, False), (wk, kt, True), (wq, qt, True)):
                    acc = ps.tile([128, 512], f32, tag="big", bufs=4, name=f"acc_{ch}")
                    for d in range(ND):
                        nc.tensor.matmul(
                            acc[:], w_sb[:, d, :],
                            xt[:, d, ch * 512:(ch + 1) * 512],
                            start=(d == 0), stop=(d == ND - 1),
                        )
                    nc.vector.tensor_copy(dst[:, ch * 512:(ch + 1) * 512], acc[:])
                for j in range(4 * ch, 4 * ch + 4):
                    tp = ps.tile([128, 128], f32, tag="otacc", bufs=2)
                    nc.tensor.transpose(tp[:], vt[:, j * 128:(j + 1) * 128], ident[:])
                    nc.vector.tensor_copy(v[:, j, :], tp[:])
                attention_chunk(ch)

    nc.compile()
    return nc


def kernel(x, W_Q, W_K, W_V):
    from concourse import bass_utils

    if "nc" not in _CACHE:
        _CACHE["nc"] = _build()
    nc = _CACHE["nc"]

    def warr(W):
        return np.ascontiguousarray(
            np.asarray(W, np.float32).reshape(ND, 128, H).transpose(1, 0, 2))

    wqr, wkr, wvr = warr(W_Q), warr(W_K), warr(W_V)
    x = np.asarray(x, np.float32)
    in_maps = [
        {"xt": np.ascontiguousarray(x[b].T), "wq": wqr, "wk": wkr, "wv": wvr}
        for b in range(B)
    ]
    _CACHE["in_maps"] = in_maps
    res = bass_utils.run_bass_kernel_spmd(nc, in_maps, core_ids=list(range(B)))
    return np.stack([res.results[b]["ot"].T for b in range(B)]).astype(np.float32)



# revision 3
# speedup vs baseline: 1.5450x; 1.5450x over previous
"""Single-head causal attention (B=8, T=2048, D=1024, H=128) on 8 TRN2 NeuronCores.

Sharding: one batch element per core (data-parallel over B).

Per-core algorithm, all matmuls in bf16 (full PE rate at any width + fast
weight load), fp32 PSUM accumulation:
  - host supplies x^T as [chunk, part, d, 512] bf16 and packed W [part, 3, d, h]
  - per 512-wide q-chunk: Q^T/K^T/V^T = W^T @ x^T via PE (8 d-tile accumulate),
    DVE-cast PSUM -> bf16 SBUF; V^T PE-transposed to V[k,h] tiles
  - attention per chunk over k-tile PAIRS: S^T[k, q] for two k-tiles into one
    2-bank PSUM tile, ONE exp over [128, <=1024] on ACT (scale folded in),
    causal mask on diagonal tiles via gpsimd affine_select (also zeroes the
    stale/garbage region), O^T += V_j @ P^T accumulated in PSUM,
    row-sum partials on DVE (bf16 pair-add + fp32 accumulate)
  - O^T (unnormalized) and the [128,512] row-sum partials are DMA'd out;
    host reduces partials and divides (cheap) - no on-chip reciprocal.
  - schedule: chunk c's attention interleaved with chunk c+1's projections to
    fill PE bubbles left by ACT latency; ACT table preloaded and PE HAM
    warmed with dummy matmuls during the initial DMA fill.
"""
import numpy as np
import ml_dtypes

B, T, D, H = 8, 2048, 1024, 128
ND = D // 128      # 8 d-tiles
NTK = T // 128     # 16 k-tiles
NCH = T // 512     # 4 q-chunks
SCALE = float(H) ** -0.5

_CACHE = {}


def _build():
    import concourse.bass as bass  # noqa: F401
    from concourse import bacc
    import concourse.mybir as mybir
    import concourse.tile as tile
    from concourse.masks import make_identity

    f32 = mybir.dt.float32
    bf16 = mybir.dt.bfloat16

    nc = bacc.Bacc("TRN2", target_bir_lowering=False)
    xt_d = nc.dram_tensor("xt", (NCH, 128, ND, 512), bf16, kind="ExternalInput")
    w_d = nc.dram_tensor("w", (128, 3, ND, H), bf16, kind="ExternalInput")
    ot_d = nc.dram_tensor("ot", (H, T), f32, kind="ExternalOutput")
    pc_d = nc.dram_tensor("pacc", (128, T), f32, kind="ExternalOutput")

    with tile.TileContext(nc) as tc:
        with (
            tc.tile_pool(name="sb", bufs=1) as sb,
            tc.tile_pool(name="ps", bufs=1, space="PSUM") as ps,
        ):
            xt = sb.tile([128, NCH, ND, 512], bf16, tag="xt")
            w = sb.tile([128, 3, ND, H], bf16, tag="w")
            qt = sb.tile([128, NCH, 512], bf16, tag="qt")
            kt = sb.tile([128, NTK, H], bf16, tag="kt")
            v = sb.tile([128, NTK, H], bf16, tag="v")
            ident = sb.tile([128, 128], bf16, tag="ident")
            warm = sb.tile([1, 16], f32, tag="warm")

            # ---- ACT exp-table preload + identity, before any DMA dependency
            nc.gpsimd.memset(warm[:], 0.0)
            nc.scalar.activation(warm[:], warm[:],
                                 mybir.ActivationFunctionType.Exp)
            make_identity(nc, ident[:])

            # ---- input DMAs (x chunk-major, 2 d-tiles per transfer) ----
            nc.sync.dma_start(w[:], w_d[:])
            for c in range(NCH):
                for g in range(0, ND, 2):
                    nc.sync.dma_start(xt[:, c, g:g + 2, :],
                                      xt_d[c, :, g:g + 2, :])

            # ---- PE warm-up: dummy matmuls during DMA fill (HAM ramp) ----
            warm_ps = ps.tile([128, 512], f32, tag="po", bufs=2, name="warmps")
            for i in range(24):
                nc.tensor.matmul(warm_ps[:, 0:128], ident[:], ident[:],
                                 start=True, stop=True)

            # ---- projection quanta for chunk c (list of thunks) ----
            def proj_quanta(c):
                items = []

                def mk_proj(wi, dst):
                    # wi: 0=Q,1=K,2=V ; dst: SBUF bf16 AP to cast into
                    acc = [None]

                    def mm(lo, hi):
                        def f():
                            if lo == 0:
                                acc[0] = ps.tile([128, 512], f32, tag="po",
                                                 bufs=2, name=f"acc{c}_{wi}")
                            for d in range(lo, hi):
                                nc.tensor.matmul(
                                    acc[0][:], w[:, wi, d, :], xt[:, c, d, :],
                                    start=(d == 0), stop=(d == ND - 1),
                                )
                        return f

                    def cast():
                        nc.vector.tensor_copy(dst, acc[0][:])
                    return [mm(0, 4), mm(4, 8), cast]

                vt = sb.tile([128, 512], bf16, tag="vt", bufs=2, name=f"vt{c}")
                items += mk_proj(0, qt[:, c, :])
                items += mk_proj(1, kt[:, 4 * c:4 * c + 4, :])
                items += mk_proj(2, vt[:])

                # V^T -> V via PE transpose (bf16, into bitcast view of a
                # po-pool tile), then one cast into v tiles
                tp = [None]

                def transp():
                    tp[0] = ps.tile([128, 512], f32, tag="po", bufs=2,
                                    name=f"tp{c}")
                    tview = tp[0][:].bitcast(bf16)
                    for s in range(4):
                        nc.tensor.transpose(
                            tview[:, 128 * s:128 * (s + 1)],
                            vt[:, 128 * s:128 * (s + 1)], ident[:])

                def vcast():
                    nc.vector.tensor_copy(v[:, 4 * c:4 * c + 4, :],
                                          tp[0][:].bitcast(bf16)[:, 0:512])
                items += [transp, vcast]
                return items

            # ---- one attention pair (k-tiles 2p, 2p+1) of chunk c ----
            def att_pair(c, p, otp, pacc):
                nk = 4 * (c + 1)
                j0, j1 = 2 * p, 2 * p + 1
                d0 = j0 >= 4 * c   # diagonal?
                d1 = j1 >= 4 * c
                lo0 = 128 * (j0 - 4 * c) if d0 else 0
                lo1 = 128 * (j1 - 4 * c) if d1 else 0

                sc = ps.tile([128, 1024], f32, tag="sc", bufs=2,
                             name=f"sc{c}_{p}")
                nc.tensor.matmul(sc[:, lo0:512], kt[:, j0, :],
                                 qt[:, c, lo0:512], start=True, stop=True)
                nc.tensor.matmul(sc[:, 512 + lo1:1024], kt[:, j1, :],
                                 qt[:, c, lo1:512], start=True, stop=True)

                pt = sb.tile([128, 1024], bf16, tag="pt", bufs=4,
                             name=f"pt{c}_{p}")
                nc.scalar.activation(pt[:, lo0:1024], sc[:, lo0:1024],
                                     mybir.ActivationFunctionType.Exp,
                                     scale=SCALE)
                for hh, dg, lo in ((0, d0, lo0), (1, d1, lo1)):
                    if dg:
                        m = (2 * p + hh) - 4 * c
                        nc.gpsimd.affine_select(
                            out=pt[:, 512 * hh:512 * (hh + 1)],
                            in_=pt[:, 512 * hh:512 * (hh + 1)],
                            compare_op=mybir.AluOpType.is_ge, fill=0.0,
                            base=-128 * m, pattern=[[1, 512]],
                            channel_multiplier=-1,
                        )

                nc.tensor.matmul(otp[:, lo0:512], v[:, j0, :],
                                 pt[:, lo0:512],
                                 start=(j0 == 0), stop=False)
                nc.tensor.matmul(otp[:, lo1:512], v[:, j1, :],
                                 pt[:, 512 + lo1:1024],
                                 start=False, stop=(j1 == nk - 1))

                # row-sum partials: bf16 pair-add, fp32 accumulate
                with nc.allow_low_precision(reason="softmax denominator"):
                    if p == 0:
                        nc.vector.tensor_add(pacc[:], pt[:, 0:512],
                                             pt[:, 512:1024])
                    else:
                        tmp = sb.tile([128, 512], bf16, tag="ptmp", bufs=2,
                                      name=f"tmp{c}_{p}")
                        nc.vector.tensor_add(tmp[:], pt[:, 0:512],
                                             pt[:, 512:1024])
                        nc.vector.tensor_add(pacc[:], pacc[:], tmp[:])

            # ---- attention chunk c as a list of thunks ----
            def att_items(c):
                npairs = 2 * (c + 1)
                otp = [None]
                pacc = [None]
                items = []

                def start():
                    otp[0] = ps.tile([128, 512], f32, tag="ot", bufs=2,
                                     name=f"ot{c}")
                    pacc[0] = sb.tile([128, 512], f32, tag="pacc", bufs=2,
                                      name=f"pacc{c}")
                items.append(start)
                for p in range(npairs):
                    items.append(lambda p=p: att_pair(c, p, otp[0], pacc[0]))

                def finish():
                    osb = sb.tile([128, 512], f32, tag="osb", bufs=2,
                                  name=f"osb{c}")
                    nc.vector.tensor_copy(osb[:], otp[0][:])
                    nc.sync.dma_start(ot_d[:, 512 * c:512 * (c + 1)], osb[:])
                    nc.sync.dma_start(pc_d[:, 512 * c:512 * (c + 1)],
                                      pacc[0][:])
                items.append(finish)
                return items

            # ---- schedule: proj(0); att(c) interleaved with proj(c+1) ----
            for it in proj_quanta(0):
                it()
            for c in range(NCH):
                att = att_items(c)
                filler = proj_quanta(c + 1) if c + 1 < NCH else []
                # att = [start, pair0..pairN-1, finish]
                pairs = att[1:-1]
                att[0]()
                nf = len(filler)
                np_ = len(pairs)
                fi = 0
                for i, pair in enumerate(pairs):
                    pair()
                    # proportionally consume filler after each pair
                    target = (i + 1) * nf // np_
                    while fi < target:
                        filler[fi]()
                        fi += 1
                att[-1]()

    nc.compile()
    return nc


def kernel(x, W_Q, W_K, W_V):
    from concourse import bass_utils

    if "nc" not in _CACHE:
        _CACHE["nc"] = _build()
    nc = _CACHE["nc"]

    bf = ml_dtypes.bfloat16

    def warr(W):
        return np.asarray(W, np.float32).reshape(ND, 128, H).transpose(1, 0, 2)

    # w layout [part, 3, d, h]
    wpack = np.ascontiguousarray(
        np.stack([warr(W_Q), warr(W_K), warr(W_V)], axis=1)).astype(bf)
    x = np.asarray(x, np.float32)
    in_maps = []
    for b in range(B):
        # xt layout [chunk, part, d, 512]:  A[c,p,d,j] = x[b][512c+j, 128d+p]
        xa = np.ascontiguousarray(
            x[b].reshape(NCH, 512, ND, 128).transpose(0, 3, 2, 1)).astype(bf)
        in_maps.append({"xt": xa, "w": wpack})
    _CACHE["in_maps"] = in_maps
    res = bass_utils.run_bass_kernel_spmd(nc, in_maps, core_ids=list(range(B)))
    out = np.empty((B, T, H), np.float32)
    for b in range(B):
        ot = res.results[b]["ot"]          # [H, T] unnormalized
        denom = res.results[b]["pacc"].sum(axis=0)   # [T]
        out[b] = (ot / denom[None, :]).T
    return out


# revision 8
# speedup vs baseline: 1.7851x; 1.1554x over previous
"""Single-head causal attention (B=8, T=2048, D=1024, H=128) on 8 TRN2 NeuronCores.

Sharding: one batch element per core (data-parallel over B).

Per-core algorithm, all matmuls bf16 (full PE rate at any width), fp32 PSUM:
  - host supplies x^T as [chunk, part, d, 512] bf16 and packed W [part, 3, d, h]
  - per 512-wide q-chunk: Q^T/K^T/V^T = W^T @ x^T (8 d-tile PSUM accumulate),
    DVE-cast to bf16 SBUF; V^T -> V[k,h] tiles via DMA-engine transpose
  - attention per chunk over k-tile PAIRS: S^T[k,q] for two k-tiles into one
    2-bank PSUM tile, ONE exp over [128,<=1024] on ACT (scale folded in),
    causal masks via gpsimd affine_select narrowed to the triangular window
    (also zeroes stale regions), O^T += V_j @ P^T accumulated in PSUM
  - row sums via a log-depth tree of wide bf16 DVE adds over the per-chunk
    P^T tile (few instructions; bf16 runs 2x on DVE), final fp32 combine
  - O^T (unnormalized, bf16) + row-sum partials (fp32) DMA'd out; host
    reduces partials and normalizes (cheap) - no on-chip reciprocal
  - schedule: chunks processed [1,2,3,0] so the cheapest chunk drains last;
    att(c) interleaved with later projections to fill PE bubbles; ACT exp
    table preloaded and PE HAM warmed with dummy matmuls during DMA fill;
    input DMA descriptors pushed from three different engine queues in
    parallel (a single queue serializes at ~0.7us per push).
"""
import numpy as np
import ml_dtypes

B, T, D, H = 8, 2048, 1024, 128
ND = D // 128      # 8 d-tiles
NTK = T // 128     # 16 k-tiles
NCH = T // 512     # 4 q-chunks
SCALE = float(H) ** -0.5

_CACHE = {}


def _build():
    import concourse.bass as bass  # noqa: F401
    from concourse import bacc
    import concourse.mybir as mybir
    import concourse.tile as tile

    f32 = mybir.dt.float32
    bf16 = mybir.dt.bfloat16

    nc = bacc.Bacc("TRN2", target_bir_lowering=False)
    xt_d = nc.dram_tensor("xt", (NCH, 128, ND, 512), bf16, kind="ExternalInput")
    w_d = nc.dram_tensor("w", (128, 3, ND, H), bf16, kind="ExternalInput")
    ot_d = nc.dram_tensor("ot", (H, T), bf16, kind="ExternalOutput")
    pc_d = nc.dram_tensor("pacc", (128, T), f32, kind="ExternalOutput")

    with tile.TileContext(nc) as tc:
        with (
            tc.tile_pool(name="sb", bufs=1) as sb,
            tc.tile_pool(name="ps", bufs=1, space="PSUM") as ps,
        ):
            xt = sb.tile([128, NCH, ND, 512], bf16, tag="xt")
            w = sb.tile([128, 3, ND, H], bf16, tag="w")
            qt = sb.tile([128, NCH, 512], bf16, tag="qt")
            kt = sb.tile([128, NTK, H], bf16, tag="kt")
            v = sb.tile([128, NTK, H], bf16, tag="v")
            wdum = sb.tile([128, 128], bf16, tag="wdum")
            warm = sb.tile([1, 16], f32, tag="warm")

            # ---- gpsimd: memsets first (unblock PE warm-up ASAP) ----
            nc.gpsimd.memset(wdum[:], 0.0)
            nc.gpsimd.memset(warm[:], 0.0)

            # ---- input DMA pushes spread across both HWDGE queues ----
            nc.scalar.dma_start(w[:], w_d[:])
            # x chunk 0 split across the two queues for earliest availability
            nc.sync.dma_start(xt[:, 0, 0:4, :], xt_d[0, :, 0:4, :])
            nc.scalar.dma_start(xt[:, 0, 4:8, :], xt_d[0, :, 4:8, :])
            nc.sync.dma_start(xt[:, 1, :, :], xt_d[1])
            nc.scalar.dma_start(xt[:, 2, :, :], xt_d[2])
            nc.sync.dma_start(xt[:, 3, :, :], xt_d[3])

            # ---- ACT exp-table preload ----
            nc.scalar.activation(warm[:], warm[:],
                                 mybir.ActivationFunctionType.Exp)

            # ---- PE warm-up (HAM ramp) on dummy weights during DMA fill ----
            warm_ps = ps.tile([128, 512], f32, tag="po", bufs=2, name="warmps")
            for i in range(34):
                nc.tensor.matmul(warm_ps[:, 0:128], wdum[:], wdum[:],
                                 start=True, stop=True)

            # ---- projection quanta for chunk c ----
            def proj_quanta(c):
                items = []

                def mk_proj(wi, dst):
                    acc = [None]

                    def mm(lo, hi):
                        def f():
                            if lo == 0:
                                acc[0] = ps.tile([128, 512], f32, tag="po",
                                                 bufs=2, name=f"acc{c}_{wi}")
                            for d in range(lo, hi):
                                nc.tensor.matmul(
                                    acc[0][:], w[:, wi, d, :], xt[:, c, d, :],
                                    start=(d == 0), stop=(d == ND - 1),
                                )
                        return f

                    def cast():
                        nc.vector.tensor_copy(dst, acc[0][:])
                    return [mm(0, 4), mm(4, 8), cast]

                vt = sb.tile([128, 512], bf16, tag="vt", bufs=2, name=f"vt{c}")
                items += mk_proj(0, qt[:, c, :])
                items += mk_proj(1, kt[:, 4 * c:4 * c + 4, :])
                items += mk_proj(2, vt[:])

                def transp():
                    for s in range(4):
                        nc.sync.dma_start_transpose(
                            v[:, 4 * c + s, :], vt[:, 128 * s:128 * (s + 1)])
                items.append(transp)
                return items

            # ---- one attention pair (k-tiles 2p, 2p+1) of chunk c ----
            def scores_exp(c, p, pt):
                j0, j1 = 2 * p, 2 * p + 1
                d0 = j0 >= 4 * c
                d1 = j1 >= 4 * c
                lo0 = 128 * (j0 - 4 * c) if d0 else 0
                lo1 = 128 * (j1 - 4 * c) if d1 else 0

                sc = ps.tile([128, 1024], f32, tag="sc", bufs=2,
                             name=f"sc{c}_{p}")
                nc.tensor.matmul(sc[:, lo0:512], kt[:, j0, :],
                                 qt[:, c, lo0:512], start=True, stop=True)
                nc.tensor.matmul(sc[:, 512 + lo1:1024], kt[:, j1, :],
                                 qt[:, c, lo1:512], start=True, stop=True)

                # exp over the full pair region; stale/masked cols are zeroed
                # by the affine_select masks below (stale scores are bounded,
                # so exp cannot overflow)
                nc.scalar.activation(
                    pt[:, p, :, :], sc[:],
                    mybir.ActivationFunctionType.Exp, scale=SCALE)
                for hh, dg, lo in ((0, d0, lo0), (1, d1, lo1)):
                    if dg:
                        m = (2 * p + hh) - 4 * c
                        wid = lo + 128
                        nc.gpsimd.affine_select(
                            out=pt[:, p, hh, 0:wid], in_=pt[:, p, hh, 0:wid],
                            compare_op=mybir.AluOpType.is_ge, fill=0.0,
                            base=-128 * m, pattern=[[1, wid]],
                            channel_multiplier=-1,
                        )

            def pv_mm(c, p, pt, otp):
                nk = 4 * (c + 1)
                j0, j1 = 2 * p, 2 * p + 1
                lo0 = 128 * (j0 - 4 * c) if j0 >= 4 * c else 0
                lo1 = 128 * (j1 - 4 * c) if j1 >= 4 * c else 0
                nc.tensor.matmul(otp[:, lo0:512], v[:, j0, :],
                                 pt[:, p, 0, lo0:512],
                                 start=(j0 == 0), stop=False)
                nc.tensor.matmul(otp[:, lo1:512], v[:, j1, :],
                                 pt[:, p, 1, lo1:512],
                                 start=False, stop=(j1 == nk - 1))

            # ---- attention chunk c: returns list of thunks ----
            def att_items(c):
                npairs = 2 * (c + 1)
                state = {}
                items = []

                def start():
                    state['ot'] = ps.tile([128, 512], f32, tag="ot", bufs=2,
                                          name=f"ot{c}")
                    state['pt'] = sb.tile([128, 8, 2, 512], bf16, tag="pt",
                                          bufs=2, name=f"pt{c}")
                    state['pacc'] = sb.tile([128, 512], f32, tag="pacc",
                                            bufs=2, name=f"pacc{c}")
                    state['t1'] = sb.tile([128, 8, 512], bf16, tag="t1",
                                          bufs=2, name=f"t1{c}")
                    state['t2'] = sb.tile([128, 4, 512], bf16, tag="t2",
                                          bufs=2, name=f"t2{c}")
                    state['t3'] = sb.tile([128, 2, 512], bf16, tag="t3",
                                          bufs=2, name=f"t3{c}")
                items.append(start)

                # tree ops keyed by "emit after pair idx" (mask of that pair)
                tree_after = {i: [] for i in range(npairs)}
                h = npairs // 2

                def lp(f, *a):
                    return lambda: f(*a)

                def add(o, i0, i1):
                    with nc.allow_low_precision(reason="softmax denominator"):
                        nc.vector.tensor_add(o, i0, i1)

                if npairs >= 4:
                    tree_after[h - 1].append(lp(
                        lambda: add(state['t1'][:, 0:h, :],
                                    state['pt'][:, 0:h, 0, :],
                                    state['pt'][:, 0:h, 1, :])))
                    tree_after[npairs - 1].append(lp(
                        lambda: add(state['t1'][:, h:npairs, :],
                                    state['pt'][:, h:npairs, 0, :],
                                    state['pt'][:, h:npairs, 1, :])))
                else:
                    tree_after[npairs - 1].append(lp(
                        lambda: add(state['t1'][:, 0:npairs, :],
                                    state['pt'][:, 0:npairs, 0, :],
                                    state['pt'][:, 0:npairs, 1, :])))

                def fold():
                    # fold t1[0:npairs] by contiguous halves down to fp32 pacc
                    t1, t2, t3 = state['t1'], state['t2'], state['t3']
                    pacc = state['pacc']
                    if npairs == 2:
                        add(pacc[:], t1[:, 0, :], t1[:, 1, :])
                    elif npairs == 4:
                        add(t2[:, 0:2, :], t1[:, 0:2, :], t1[:, 2:4, :])
                        add(pacc[:], t2[:, 0, :], t2[:, 1, :])
                    elif npairs == 6:
                        add(t2[:, 0:3, :], t1[:, 0:3, :], t1[:, 3:6, :])
                        add(t3[:, 0:1, :], t2[:, 0:1, :], t2[:, 1:2, :])
                        add(pacc[:], t3[:, 0, :], t2[:, 2, :])
                    else:  # 8
                        add(t2[:, 0:4, :], t1[:, 0:4, :], t1[:, 4:8, :])
                        add(t3[:, 0:2, :], t2[:, 0:2, :], t2[:, 2:4, :])
                        add(pacc[:], t3[:, 0, :], t3[:, 1, :])
                tree_after[npairs - 1].append(fold)

                for p in range(npairs):
                    def pair_step(p=p):
                        scores_exp(c, p, state['pt'])
                        if p > 0:
                            pv_mm(c, p - 1, state['pt'], state['ot'])
                        for f in tree_after.get(p - 1, ()):
                            f()
                    items.append(pair_step)

                def last():
                    pv_mm(c, npairs - 1, state['pt'], state['ot'])
                    for f in tree_after[npairs - 1]:
                        f()
                    osb = sb.tile([128, 512], bf16, tag="osb", bufs=2,
                                  name=f"osb{c}")
                    nc.vector.tensor_copy(osb[:], state['ot'][:])
                    nc.sync.dma_start(ot_d[:, 512 * c:512 * (c + 1)], osb[:])
                    nc.sync.dma_start(pc_d[:, 512 * c:512 * (c + 1)],
                                      state['pacc'][:])
                items.append(last)
                return items

            # ---- schedule ----
            def run_interleaved(att, filler):
                att[0]()
                pairs = att[1:-1]
                nf, np_ = len(filler), len(pairs)
                fi = 0
                for i, pair in enumerate(pairs):
                    pair()
                    target = (i + 1) * nf // np_
                    while fi < target:
                        filler[fi]()
                        fi += 1
                att[-1]()

            for it in proj_quanta(0):
                it()
            for it in proj_quanta(1):
                it()
            run_interleaved(att_items(1), proj_quanta(2))
            run_interleaved(att_items(2), proj_quanta(3))
            run_interleaved(att_items(3), [])
            run_interleaved(att_items(0), [])

    nc.compile()
    return nc


def kernel(x, W_Q, W_K, W_V):
    from concourse import bass_utils

    if "nc" not in _CACHE:
        _CACHE["nc"] = _build()
    nc = _CACHE["nc"]

    bf = ml_dtypes.bfloat16

    def warr(W):
        return np.asarray(W, np.float32).reshape(ND, 128, H).transpose(1, 0, 2)

    wpack = np.ascontiguousarray(
        np.stack([warr(W_Q), warr(W_K), warr(W_V)], axis=1)).astype(bf)
    x = np.asarray(x, np.float32)
    in_maps = []
    for b in range(B):
        # xt layout [chunk, part, d, 512]:  A[c,p,d,j] = x[b][512c+j, 128d+p]
        xa = np.ascontiguousarray(
            x[b].reshape(NCH, 512, ND, 128).transpose(0, 3, 2, 1)).astype(bf)
        in_maps.append({"xt": xa, "w": wpack})
    _CACHE["in_maps"] = in_maps
    res = bass_utils.run_bass_kernel_spmd(nc, in_maps, core_ids=list(range(B)))
    out = np.empty((B, T, H), np.float32)
    for b in range(B):
        ot = np.asarray(res.results[b]["ot"], dtype=np.float32)  # [H, T]
        denom = res.results[b]["pacc"].sum(axis=0)               # [T]
        out[b] = (ot / denom[None, :]).T
    return out
